# revision 2
# baseline (speedup 1.0000x reference)
import sys

sys.path.insert(0, "/opt/trn_rl_repo")
import numpy as np
import ml_dtypes
import concourse.bass as bass
import concourse.mybir as mybir
import concourse.tile as tile
from concourse.bass_utils import run_bass_kernel_spmd

F32 = mybir.dt.float32
BF16 = mybir.dt.bfloat16
F8 = mybir.dt.float8e4
AF = mybir.ActivationFunctionType
ALU = mybir.AluOpType
DR = mybir.MatmulPerfMode.DoubleRow

C = 512
NH = 4          # heads per core (8 global, split in 2 groups of 4)
HD = 64
THETA = 10.0

import json as _json
import concourse.bass2jax as _b2j
import concourse.bass_utils as _bu

_ORIG_COMPILE = _bu.compile_bir_kernel


def _patched_compile_bir_kernel(bir_json, tmpdir, neff_name="file.neff"):
    """Walrus rejects instructions whose sync waits+updates exceed 2.
    Rewrite the BIR: move excess waits onto inserted same-engine Drains."""
    d = _json.loads(bir_json)
    for fn in d.get("functions", []):
        for b in fn.get("blocks", []):
            out = []
            for i in b.get("instructions", []):
                si = i.get("sync_info")
                if si:
                    ow = si.get("on_wait") or []
                    ou = si.get("on_update") or []
                    cap = 1 if i.get("opcode") in ("Drain", "Ldweights") else 2
                    budget = cap - len(ou)
                    if len(ow) > budget:
                        keep = ow[-budget:] if budget > 0 else []
                        extra = ow[:-budget] if budget > 0 else ow
                        for ci, w in enumerate(extra):
                            out.append({
                                "debug": i.get("debug", 0),
                                "engine": i["engine"],
                                "ins": [], "outs": [],
                                "name": f"{i['name']}sw{ci}",
                                "opcode": "Drain",
                                "sync_info": {"on_update": [],
                                              "on_wait": [w]},
                            })
                        si["on_wait"] = keep
                out.append(i)
            b["instructions"] = out
    return _ORIG_COMPILE(_json.dumps(d).encode(), tmpdir, neff_name=neff_name)


_bu.compile_bir_kernel = _patched_compile_bir_kernel
_b2j.compile_bir_kernel = _patched_compile_bir_kernel


def _build_nc(n_tok):
    nspan = n_tok // 512
    nc = bass.Bass()
    xT8 = nc.declare_dram_parameter("xT8", [128, nspan * 2048], F8, isOutput=False)
    w_q = nc.declare_dram_parameter("w_q", [4, 128, 256], F8, isOutput=False)
    w_kv = nc.declare_dram_parameter("w_kv", [4, 128, 512], F8, isOutput=False)
    bq = nc.declare_dram_parameter("bq", [128, 2], F32, isOutput=False)
    bkrow = nc.declare_dram_parameter("bkrow", [1, 512], BF16, isOutput=False)
    wp = nc.declare_dram_parameter("wp", [2, 128, 512], F8, isOutput=False)
    # per-span rope tables: q [c;-s] and [s;c]; k paired [c|-s] and [s|c]
    # (all pre-scaled 1/64 to undo the x64 fp8 weight scale)
    qtab = nc.declare_dram_parameter("qtab", [128, nspan * 1024], BF16, isOutput=False)
    qtab2 = nc.declare_dram_parameter("qtab2", [128, nspan * 1024], BF16, isOutput=False)
    kt1 = nc.declare_dram_parameter("kt1", [128, nspan * 1024], BF16, isOutput=False)
    kt2 = nc.declare_dram_parameter("kt2", [128, nspan * 1024], BF16, isOutput=False)
    sel = nc.declare_dram_parameter("sel", [4, 16, 2, 128], BF16, isOutput=False)
    mask0 = nc.declare_dram_parameter("mask0", [128, 128], BF16, isOutput=False)
    mask1 = nc.declare_dram_parameter("mask1", [128, 128], BF16, isOutput=False)
    dmask = nc.declare_dram_parameter("dmask", [4, 128, 16], BF16, isOutput=False)
    y = nc.declare_dram_parameter("y", [n_tok, 512], BF16, isOutput=True)

    with nc.allow_low_precision(reason="fp8 pipeline by design"), tile.TileContext(nc) as tc:
        with tc.tile_pool(name="wpool", bufs=1) as wpool, \
             tc.tile_pool(name="store", bufs=1) as store:
            # ---- persistent tiles ----
            wq_t = wpool.tile([128, 4, 256], F8, name="wq")
            wkv_t = wpool.tile([128, 4, 512], F8, name="wkv")
            bq_t = wpool.tile([128, 2], F32, name="bq")
            bk_t = wpool.tile([1, 512], BF16, name="bk")
            wp_t = wpool.tile([128, 2, 512], F8, name="wp")
            sel_t = wpool.tile([16, 4, 2, 128], BF16, name="sel")
            mask0_t = wpool.tile([128, 128], BF16, name="mask0")
            mask1_t = wpool.tile([128, 128], BF16, name="mask1")
            dmask_t = wpool.tile([128, 4, 16], BF16, name="dmask")
            ones_t = wpool.tile([1, 512], BF16, name="ones")

            nc.sync.dma_start(wq_t[:], w_q.rearrange("c p d -> p c d"))
            nc.sync.dma_start(wkv_t[:], w_kv.rearrange("c p d -> p c d"))
            nc.sync.dma_start(bq_t[:], bq[:])
            nc.sync.dma_start(bk_t[:], bkrow[:])
            nc.sync.dma_start(wp_t[:], wp.rearrange("e p c -> p e c"))
            nc.sync.dma_start(sel_t[:], sel.rearrange("j p i d -> p j i d"))
            nc.sync.dma_start(mask0_t[:], mask0[:])
            nc.sync.dma_start(mask1_t[:], mask1[:])
            nc.sync.dma_start(dmask_t[:], dmask.rearrange("j p c -> p j c"))
            nc.vector.memset(ones_t[:], 1.0)

            # q features for the whole sequence: [128 d, 2 (R/I), n_tok] fp8
            # span-major so each span's [128, 2, 512] slice is a dense window
            # (a 2*n_tok-strided middle dim defeats DoubleRow rhs packing)
            q_store = store.tile([128, n_tok // 512, 2, 512], BF16, name="qs")

            # kv lhsT (fp8, DoubleRow R/I pairs) + den lhsT (4 span-phase
            # variants: variant j writes only den columns 4j..4j+3)
            lkv = [wpool.tile([128, 2, 128], BF16, name=f"lkv{i}") for i in range(2)]
            dm8 = wpool.tile([128, 4, 2, 16], BF16, name="dm8")

            # ================ pass 1 ================
            with tc.tile_pool(name="kvacc", bufs=1, space="PSUM") as kvacc:
                kvR = kvacc.tile([128, 257], F32, name="kvR")
                kvI = kvacc.tile([128, 257], F32, name="kvI")
                with tc.tile_pool(name="p1", bufs=3) as p1, \
                     tc.tile_pool(name="kfp", bufs=4) as kfp, \
                     tc.tile_pool(name="xp", bufs=3) as xp, \
                     tc.tile_pool(name="qps", bufs=1, space="PSUM") as qps, \
                     tc.tile_pool(name="kvps", bufs=1, space="PSUM") as kvps:
                    kf_hist = [None, None, None]
                    va_hist = [None, None, None, None]
                    rot_hist = [None, None, None]
                    xt_tiles = {}
                    tab_tiles = {}

                    def dma_xt(s):
                        xt = xp.tile([128, 4, 512], F8, name="xt")
                        nc.sync.dma_start(
                            xt[:], xT8[:, s * 2048:(s + 1) * 2048].rearrange("p (c t) -> p c t", c=4))
                        xt_tiles[s] = xt

                    def dma_tab(s):
                        tq = p1.tile([128, 2, 2, 512], BF16, name="tq")
                        tk = p1.tile([128, 2, 4, 256], BF16, name="tk")
                        nc.sync.dma_start(
                            tq[:, 0, :, :], qtab[:, s * 1024:(s + 1) * 1024].rearrange("p (b t) -> p b t", b=2))
                        nc.sync.dma_start(
                            tq[:, 1, :, :], qtab2[:, s * 1024:(s + 1) * 1024].rearrange("p (b t) -> p b t", b=2))
                        nc.sync.dma_start(
                            tk[:, 0, :, :], kt1[:, s * 1024:(s + 1) * 1024].rearrange("p (t d) -> p t d", t=4))
                        nc.sync.dma_start(
                            tk[:, 1, :, :], kt2[:, s * 1024:(s + 1) * 1024].rearrange("p (t d) -> p t d", t=4))
                        tab_tiles[s] = (tq, tk)

                    def emit_kv(s):
                        kf = kf_hist[s % 3]
                        va = va_hist[s % 4]
                        for t in range(4):
                            st = (s == 0 and t == 0)
                            sp = (s == nspan - 1 and t == 3)
                            nc.tensor.matmul(kvR[:], kf[:, 0, t, :], va[:, t, :],
                                             start=st, stop=sp)
                            nc.tensor.matmul(kvI[:], kf[:, 1, t, :], va[:, t, :],
                                             start=st, stop=sp)

                    # prefetch: x tiles 2 spans ahead, tables 1 span ahead
                    dma_xt(0)
                    dma_xt(1)
                    dma_tab(0)
                    for s in range(nspan):
                        if s + 2 < nspan:
                            dma_xt(s + 2)
                        if s + 1 < nspan:
                            dma_tab(s + 1)
                        xt = xt_tiles.pop(s)
                        tq, tk = tab_tiles.pop(s)

                        # q matmuls: out [128 d, 2 blk, 512 tok], DoubleRow over
                        # c-chunk pairs (bias added on the scalar eviction)
                        q_ps = qps.tile([128, 2, 512], F32, name="qp")
                        for blk in range(2):
                            for j in range(2):
                                nc.tensor.matmul(
                                    q_ps[:, blk, :],
                                    wq_t[:, 2 * j:2 * j + 2, blk * 128:(blk + 1) * 128],
                                    xt[:, 2 * j:2 * j + 2, :],
                                    start=(j == 0), stop=(j == 1), perf_mode=DR)
                        # k+v matmuls: out [128 tok, 512 (kR kI v)] per t-tile;
                        # the 4 bf16 k-bias matmuls run as one batch AFTER all
                        # DR matmuls (interleaving bf16 into the fp8 stream
                        # breaks DoubleRow packing: 216 -> 318+ ns/MM)
                        kv_ps = kvps.tile([128, 4, 512], F32, name="kvp")
                        for t in range(4):
                            for j in range(2):
                                nc.tensor.matmul(
                                    kv_ps[:, t, :],
                                    xt[:, 2 * j:2 * j + 2, t * 128:(t + 1) * 128],
                                    wkv_t[:, 2 * j:2 * j + 2, :],
                                    start=(j == 0), stop=False, perf_mode=DR)
                        for t in range(4):
                            nc.tensor.matmul(kv_ps[:, t, :], ones_t[0:1, 0:128],
                                             bk_t[:], start=False, stop=True)
                        # kv accumulation, three spans back
                        if s > 2:
                            emit_kv(s - 3)

                        # ---- evict q to SBUF (scalar, bias folded in) ----
                        q_sb = p1.tile([128, 1, 2, 512], BF16, name="qsb")
                        for blk in range(2):
                            nc.scalar.activation(q_sb[:, 0, blk, :], q_ps[:, blk, :],
                                                 AF.Identity,
                                                 bias=bq_t[:, blk:blk + 1])
                        # ---- evict k to SBUF (scalar) ----
                        k_sb = p1.tile([128, 1, 4, 256], BF16, name="ksb")
                        nc.scalar.copy(k_sb[:, 0, :, :], kv_ps[:, :, 0:256])

                        # ---- va: v * 2^-9 -> fp8 (w-scale 1/64 * v-scale 1/8)
                        va = kfp.tile([128, 4, 257], BF16, name="va")
                        if s < 4:
                            nc.vector.memset(va[:, :, 256:257], 1.0 / 128.0)
                        nc.scalar.activation(va[:, :, 0:256], kv_ps[:, :, 256:512],
                                             AF.Identity, scale=1.0 / 512.0)

                        # ---- rope products, both rotations in one op ----
                        # P[:,0] = q*[c;-s], P[:,1] = q*[s;c]; same for k
                        P = p1.tile([128, 2, 2, 512], BF16, name="P")
                        nc.vector.tensor_tensor(
                            P[:], q_sb.broadcast_to([128, 2, 2, 512]), tq[:], op=ALU.mult)
                        tkP = p1.tile([128, 2, 4, 256], BF16, name="tkP")
                        nc.vector.tensor_tensor(
                            tkP[:], k_sb.broadcast_to([128, 2, 4, 256]), tk[:], op=ALU.mult)

                        # ---- combine rope halves ----
                        # rot layout: [128, 4, 512] = qR' qI' kR' kI'
                        rot = p1.tile([128, 4, 512], BF16, name="rot")
                        nc.vector.tensor_tensor(rot[:, 0:2, :], P[:, :, 0, :],
                                                P[:, :, 1, :], op=ALU.add)
                        nc.vector.tensor_tensor(
                            rot[:, 2:4, :].rearrange("p a (t d) -> p a t d", d=128),
                            tkP[:, :, :, 0:128], tkP[:, :, :, 128:256], op=ALU.add)
                        a_t = p1.tile([128, 4, 512], BF16, name="at")
                        nc.vector.tensor_scalar(a_t[:], rot[:], 1.0, 1.0,
                                                op0=ALU.add, op1=ALU.max)
                        rot_hist[s % 3] = (rot, a_t)

                        # ---- feature map elu(x)+1 = min(exp(x), max(x+1,1)),
                        # pipelined one span back so vector never waits on exp
                        def emit_feat(sf):
                            rot_f, a_f = rot_hist[sf % 3]
                            exp_t = p1.tile([128, 4, 512], BF16, name="expt")
                            nc.scalar.activation(exp_t[:], rot_f[:], AF.Exp)
                            nc.vector.tensor_tensor(
                                q_store[:, sf, :, :],
                                exp_t[:, 0:2, :], a_f[:, 0:2, :], op=ALU.min)
                            kf = kfp.tile([128, 2, 4, 128], BF16, name="kf")
                            nc.vector.tensor_tensor(
                                kf.rearrange("p r t d -> p r (t d)"),
                                exp_t[:, 2:4, :], a_f[:, 2:4, :], op=ALU.min)
                            kf_hist[sf % 3] = kf

                        if s > 1:
                            emit_feat(s - 2)
                        va_hist[s % 4] = va
                    emit_feat(nspan - 2)
                    emit_feat(nspan - 1)
                    emit_kv(nspan - 3)
                    emit_kv(nspan - 2)
                    emit_kv(nspan - 1)

                # ---- extract block-diag kv lhsT (fp8 R/I pairs) + den lhsT ----
                nc.vector.tensor_tensor(lkv[0][:, 0, :], kvR[:, 0:128], mask0_t[:], op=ALU.mult)
                nc.vector.tensor_tensor(lkv[0][:, 1, :], kvI[:, 0:128], mask0_t[:], op=ALU.mult)
                nc.vector.tensor_tensor(lkv[1][:, 0, :], kvR[:, 128:256], mask1_t[:], op=ALU.mult)
                nc.vector.tensor_tensor(lkv[1][:, 1, :], kvI[:, 128:256], mask1_t[:], op=ALU.mult)
                for j in range(4):
                    nc.vector.tensor_tensor(
                        dm8[:, j, 0, :], kvR[:, 256:257].broadcast_to([128, 16]),
                        dmask_t[:, j, :], op=ALU.mult)
                    nc.vector.tensor_tensor(
                        dm8[:, j, 1, :], kvI[:, 256:257].broadcast_to([128, 16]),
                        dmask_t[:, j, :], op=ALU.mult)

            # ===== pass 2 =====
            with tc.tile_pool(name="p2", bufs=3) as p2, \
                 tc.tile_pool(name="zt", bufs=2) as zt, \
                 tc.tile_pool(name="dps", bufs=2, space="PSUM") as dps, \
                 tc.tile_pool(name="ops", bufs=1, space="PSUM") as ops, \
                 tc.tile_pool(name="zps", bufs=1, space="PSUM") as zps, \
                 tc.tile_pool(name="yps", bufs=2, space="PSUM") as yps:
                den_tiles = {}
                zinv_tiles = {}
                st = {}

                def stage_den(s):
                    # den for span s at partitions 4*(s%4); group of 4 spans
                    # shares one PSUM bank via span-phase mask variants
                    g, j = s // 4, s % 4
                    if j == 0:
                        den_tiles[g] = dps.tile([16, 512], F32, name="denp")
                    nc.tensor.matmul(den_tiles[g][:], dm8[:, j, 0, :],
                                     q_store[:, s, 0, :],
                                     start=(j == 0), stop=False)
                    nc.tensor.matmul(den_tiles[g][:], dm8[:, j, 1, :],
                                     q_store[:, s, 1, :],
                                     start=False, stop=(j == 3))

                def stage_zinv(g):
                    # 1/z = exp(-ln(z)) for 4 spans at once
                    den_ps = den_tiles.pop(g)
                    lnz = p2.tile([16, 512], F32, name="lnz")
                    nc.scalar.activation(lnz[:], den_ps[:], AF.Ln)
                    zi = zt.tile([16, 512], BF16, name="zinv")
                    nc.scalar.activation(zi[:], lnz[:], AF.Exp, scale=-1.0)
                    zinv_tiles[g] = zi

                def stage_a(s):
                    g, j = s // 4, s % 4
                    out_ps = ops.tile([128, 2, 512], F32, name="outp")
                    for i in range(2):
                        nc.tensor.matmul(out_ps[:, i, :], lkv[i][:, 0, :],
                                         q_store[:, s, 0, :],
                                         start=True, stop=False)
                        nc.tensor.matmul(out_ps[:, i, :], lkv[i][:, 1, :],
                                         q_store[:, s, 1, :],
                                         start=False, stop=True)
                    zb_ps = zps.tile([128, 2, 512], F32, name="zbp")
                    zi = zinv_tiles[g]
                    for i in range(2):
                        nc.tensor.matmul(zb_ps[:, i, :], sel_t[:, j, i, :],
                                         zi[:], start=True, stop=True)
                    zb_sb = p2.tile([128, 2, 512], BF16, name="zbs")
                    nc.scalar.copy(zb_sb[:], zb_ps[:])
                    outT = p2.tile([128, 2, 512], F8, name="outT")
                    nc.vector.tensor_tensor(outT[:], out_ps[:], zb_sb[:], op=ALU.mult)
                    st[s] = outT

                def stage_c(s):
                    outT = st.pop(s)
                    for t in range(4):
                        y_ps = yps.tile([128, 512], F32, name="yp")
                        nc.tensor.matmul(y_ps[:],
                                         outT[:, :, t * 128:(t + 1) * 128],
                                         wp_t[:], start=True, stop=True,
                                         perf_mode=DR)
                        y_sb = p2.tile([128, 512], BF16, name="ysb")
                        if t % 2 == 0:
                            nc.scalar.copy(y_sb[:], y_ps[:])
                        else:
                            nc.vector.tensor_copy(y_sb[:], y_ps[:])
                        nc.sync.dma_start(
                            y[s * 512 + t * 128:s * 512 + (t + 1) * 128, :],
                            y_sb[:])

                for s in range(8):
                    stage_den(s)
                stage_zinv(0)
                for s in range(nspan):
                    if s >= 2:
                        stage_c(s - 2)
                    if s + 8 < nspan:
                        stage_den(s + 8)
                    if s % 4 == 0 and s + 4 < nspan:
                        stage_zinv(s // 4 + 1)
                    stage_a(s)
                stage_c(nspan - 2)
                stage_c(nspan - 1)

    return nc


_NC_CACHE = {}


def _get_nc(n_tok):
    if n_tok not in _NC_CACHE:
        _NC_CACHE[n_tok] = _build_nc(n_tok)
    return _NC_CACHE[n_tok]


def _bf(a):
    return np.ascontiguousarray(np.asarray(a, dtype=np.float32)).astype(ml_dtypes.bfloat16)


def _f8(a, scale=1.0):
    return np.ascontiguousarray(
        np.asarray(a, dtype=np.float32) * scale).astype(ml_dtypes.float8_e4m3)


_TABLES_CACHE = {}


def _tables(nspan):
    """Per-span rope tables (x1/64 to undo the x64 fp8 weight scale).

    qtab [128 d, nspan, 2, 512]: [cos ; -sin] in qT layout.
    ktc/kts [128 tok, nspan, 4 t, 128 d]: cos/sin in token layout.
    """
    if nspan in _TABLES_CACHE:
        return _TABLES_CACHE[nspan]
    j = np.arange(16)
    freqs = (1.0 / (THETA ** (4.0 * j / HD))).astype(np.float64)
    fcol = np.tile(freqs, 4)                      # [64] head-major 4h x 16f
    tx = np.arange(128).astype(np.float64)
    angx = np.outer(fcol, tx)                     # [64 f, 128 tx]
    rows = np.arange(nspan * 4).astype(np.float64)
    angy = np.outer(fcol, rows)                   # [64 f, nspan*4]

    # q tables [128, nspan, 4, 128]
    qc = np.empty((128, nspan, 4, 128), np.float32)
    qs_ = np.empty((128, nspan, 4, 128), np.float32)
    qc[0:64] = np.cos(angx)[:, None, None, :]
    qs_[0:64] = np.sin(angx)[:, None, None, :]
    qc[64:128] = np.cos(angy).reshape(64, nspan, 4, 1)
    qs_[64:128] = np.sin(angy).reshape(64, nspan, 4, 1)
    qcf = qc.reshape(128, nspan, 1, 512)
    qsf = qs_.reshape(128, nspan, 1, 512)
    qtab = np.concatenate([qcf, -qsf], axis=2).reshape(128, nspan * 1024) / 64.0
    qtab2 = np.concatenate([qsf, qcf], axis=2).reshape(128, nspan * 1024) / 64.0

    # k tables [128 tok(part), nspan, 4 t, 128 d], paired [c|-s] and [s|c]
    kc = np.empty((128, nspan, 4, 128), np.float32)
    ks = np.empty((128, nspan, 4, 128), np.float32)
    kc[:, :, :, 0:64] = np.cos(angx).T[:, None, None, :]
    ks[:, :, :, 0:64] = np.sin(angx).T[:, None, None, :]
    kc[:, :, :, 64:128] = np.cos(angy).T.reshape(1, nspan, 4, 64)
    ks[:, :, :, 64:128] = np.sin(angy).T.reshape(1, nspan, 4, 64)
    kt1 = np.concatenate([kc, -ks], axis=3).reshape(128, nspan * 1024) / 64.0
    kt2 = np.concatenate([ks, kc], axis=3).reshape(128, nspan * 1024) / 64.0
    out = (_bf(qtab), _bf(qtab2), _bf(kt1), _bf(kt2))
    _TABLES_CACHE[nspan] = out
    return out


def kernel(x, w_qkv, b_qkv, w_proj, b_proj, height, width):
    x = np.asarray(x); w_qkv = np.asarray(w_qkv); b_qkv = np.asarray(b_qkv)
    w_proj = np.asarray(w_proj); b_proj = np.asarray(b_proj)
    b, n, c = x.shape
    nc = _get_nc(n)
    qtab, qtab2, kt1, kt2 = _tables(n // 512)

    # span-phase variants: variant j reads zinv rows 4j..4j+3 / writes den
    # columns 4j..4j+3 (4 spans share one den PSUM bank / zinv tile)
    sel = np.zeros((4, 16, 2, 128), np.float32)
    for j in range(4):
        for i in range(2):
            for hf in range(2):
                sel[j, 4 * j + 2 * i + hf, i, 64 * hf:64 * hf + 64] = 1.0
    # head of partition p in the d layout: (p % 64) // 16
    hop = (np.arange(128) % 64) // 16
    mask0 = (hop[:, None] == (np.arange(128) // 64)[None, :]).astype(np.float32)
    mask1 = (hop[:, None] == (2 + np.arange(128) // 64)[None, :]).astype(np.float32)
    dmask = np.zeros((4, 128, 16), np.float32)
    for j in range(4):
        for h in range(4):
            dmask[j, :, 4 * j + h] = (hop == h)

    in_maps = []
    for core in range(8):
        bi, hg = core // 2, core % 2
        heads = [hg * NH + j for j in range(NH)]
        q0 = [h * HD + 2 * j for h in heads for j in range(16)] + \
             [h * HD + 32 + 2 * j for h in heads for j in range(16)]
        q1 = [cc + 1 for cc in q0]
        kR = [512 + cc for cc in q0]
        kI = [512 + cc for cc in q1]
        vc = [1024 + h * HD + e for h in heads for e in range(HD)]
        wq_cols = q0 + q1
        wkv_cols = kR + kI + vc
        bk = np.concatenate([b_qkv[kR + kI] * 64.0, np.zeros(256, np.float32)])
        in_maps.append({
            "xT8": _f8(x[bi].T.reshape(4, 128, n // 512, 512).transpose(1, 2, 0, 3).reshape(128, -1)),
            "w_q": _f8(w_qkv[:, wq_cols], 64.0).reshape(4, 128, 256),
            "w_kv": _f8(w_qkv[:, wkv_cols], 64.0).reshape(4, 128, 512),
            "bq": np.stack([b_qkv[q0], b_qkv[q1]], axis=1).astype(np.float32) * 64.0,
            "bkrow": _bf(bk)[None, :],
            "wp": _f8(np.stack([w_proj[hg * 256:hg * 256 + 128, :],
                                w_proj[hg * 256 + 128:hg * 256 + 256, :]]), 64.0),
            "qtab": qtab, "qtab2": qtab2, "kt1": kt1, "kt2": kt2,
            "sel": _bf(sel),
            "mask0": _bf(mask0), "mask1": _bf(mask1), "dmask": _bf(dmask),
        })
    res = run_bass_kernel_spmd(nc, in_maps, list(range(8)), trace=False)
    bias_eff = (b_proj.astype(np.float32)
                + b_qkv[1024:].astype(np.float32) @ w_proj.astype(np.float32))
    out = np.empty((b, n, c), np.float32)
    for bi in range(b):
        out[bi] = ((res.results[2 * bi]["y"].astype(np.float32)
                    + res.results[2 * bi + 1]["y"].astype(np.float32)) * (2.0 ** -10)
                   + bias_eff[None, :])
    return out


# revision 3
# speedup vs baseline: 1.0181x; 1.0181x over previous
import sys

sys.path.insert(0, "/opt/trn_rl_repo")
import numpy as np
import ml_dtypes
import concourse.bass as bass
import concourse.mybir as mybir
import concourse.tile as tile
from concourse.bass_utils import run_bass_kernel_spmd

F32 = mybir.dt.float32
BF16 = mybir.dt.bfloat16
F8 = mybir.dt.float8e4
AF = mybir.ActivationFunctionType
ALU = mybir.AluOpType
DR = mybir.MatmulPerfMode.DoubleRow

C = 512
NH = 4          # heads per core (8 global, split in 2 groups of 4)
HD = 64
THETA = 10.0

import json as _json
import concourse.bass2jax as _b2j
import concourse.bass_utils as _bu

_ORIG_COMPILE = _bu.compile_bir_kernel


def _patched_compile_bir_kernel(bir_json, tmpdir, neff_name="file.neff"):
    """Walrus rejects instructions whose sync waits+updates exceed 2.
    Rewrite the BIR: move excess waits onto inserted same-engine Drains."""
    d = _json.loads(bir_json)
    for fn in d.get("functions", []):
        for b in fn.get("blocks", []):
            out = []
            for i in b.get("instructions", []):
                si = i.get("sync_info")
                if si:
                    ow = si.get("on_wait") or []
                    ou = si.get("on_update") or []
                    cap = 1 if i.get("opcode") in ("Drain", "Ldweights") else 2
                    budget = cap - len(ou)
                    if len(ow) > budget:
                        keep = ow[-budget:] if budget > 0 else []
                        extra = ow[:-budget] if budget > 0 else ow
                        for ci, w in enumerate(extra):
                            out.append({
                                "debug": i.get("debug", 0),
                                "engine": i["engine"],
                                "ins": [], "outs": [],
                                "name": f"{i['name']}sw{ci}",
                                "opcode": "Drain",
                                "sync_info": {"on_update": [],
                                              "on_wait": [w]},
                            })
                        si["on_wait"] = keep
                out.append(i)
            b["instructions"] = out
    return _ORIG_COMPILE(_json.dumps(d).encode(), tmpdir, neff_name=neff_name)


_bu.compile_bir_kernel = _patched_compile_bir_kernel
_b2j.compile_bir_kernel = _patched_compile_bir_kernel


def _build_nc(n_tok):
    nspan = n_tok // 512
    nc = bass.Bass()
    xT8 = nc.declare_dram_parameter("xT8", [128, nspan * 2048], F8, isOutput=False)
    w_q = nc.declare_dram_parameter("w_q", [4, 128, 256], F8, isOutput=False)
    w_kv = nc.declare_dram_parameter("w_kv", [4, 128, 512], F8, isOutput=False)
    bq = nc.declare_dram_parameter("bq", [128, 2], F32, isOutput=False)
    bkrow = nc.declare_dram_parameter("bkrow", [1, 512], BF16, isOutput=False)
    wp = nc.declare_dram_parameter("wp", [2, 128, 512], BF16, isOutput=False)
    # per-span rope tables: q [c;-s] and [s;c]; k paired [c|-s] and [s|c]
    # (all pre-scaled 1/64 to undo the x64 fp8 weight scale)
    qtab = nc.declare_dram_parameter("qtab", [128, nspan * 1024], BF16, isOutput=False)
    qtab2 = nc.declare_dram_parameter("qtab2", [128, nspan * 1024], BF16, isOutput=False)
    kt1 = nc.declare_dram_parameter("kt1", [128, nspan * 1024], BF16, isOutput=False)
    kt2 = nc.declare_dram_parameter("kt2", [128, nspan * 1024], BF16, isOutput=False)
    sel = nc.declare_dram_parameter("sel", [4, 16, 2, 128], BF16, isOutput=False)
    mask0 = nc.declare_dram_parameter("mask0", [128, 128], BF16, isOutput=False)
    mask1 = nc.declare_dram_parameter("mask1", [128, 128], BF16, isOutput=False)
    dmask = nc.declare_dram_parameter("dmask", [4, 128, 16], BF16, isOutput=False)
    y = nc.declare_dram_parameter("y", [n_tok, 512], BF16, isOutput=True)

    with nc.allow_low_precision(reason="fp8 pipeline by design"), tile.TileContext(nc) as tc:
        with tc.tile_pool(name="wpool", bufs=1) as wpool, \
             tc.tile_pool(name="store", bufs=1) as store:
            # ---- persistent tiles ----
            wq_t = wpool.tile([128, 4, 256], F8, name="wq")
            wkv_t = wpool.tile([128, 4, 512], F8, name="wkv")
            bq_t = wpool.tile([128, 2], F32, name="bq")
            bk_t = wpool.tile([1, 512], BF16, name="bk")
            wp_t = wpool.tile([128, 2, 512], BF16, name="wp")
            sel_t = wpool.tile([16, 4, 2, 128], BF16, name="sel")
            mask0_t = wpool.tile([128, 128], BF16, name="mask0")
            mask1_t = wpool.tile([128, 128], BF16, name="mask1")
            dmask_t = wpool.tile([128, 4, 16], BF16, name="dmask")
            ones_t = wpool.tile([1, 512], BF16, name="ones")

            nc.sync.dma_start(wq_t[:], w_q.rearrange("c p d -> p c d"))
            nc.sync.dma_start(wkv_t[:], w_kv.rearrange("c p d -> p c d"))
            nc.sync.dma_start(bq_t[:], bq[:])
            nc.sync.dma_start(bk_t[:], bkrow[:])
            nc.sync.dma_start(wp_t[:], wp.rearrange("e p c -> p e c"))
            nc.sync.dma_start(sel_t[:], sel.rearrange("j p i d -> p j i d"))
            nc.sync.dma_start(mask0_t[:], mask0[:])
            nc.sync.dma_start(mask1_t[:], mask1[:])
            nc.sync.dma_start(dmask_t[:], dmask.rearrange("j p c -> p j c"))
            nc.vector.memset(ones_t[:], 1.0)

            # q features for the whole sequence: [128 d, 2 (R/I), n_tok] fp8
            # span-major so each span's [128, 2, 512] slice is a dense window
            # (a 2*n_tok-strided middle dim defeats DoubleRow rhs packing)
            q_store = store.tile([128, n_tok // 512, 2, 512], BF16, name="qs")

            # kv lhsT (fp8, DoubleRow R/I pairs) + den lhsT (4 span-phase
            # variants: variant j writes only den columns 4j..4j+3)
            lkv = [wpool.tile([128, 2, 128], BF16, name=f"lkv{i}") for i in range(2)]
            dm8 = wpool.tile([128, 4, 2, 16], BF16, name="dm8")

            # ================ pass 1 ================
            with tc.tile_pool(name="kvacc", bufs=1, space="PSUM") as kvacc:
                kvR = kvacc.tile([128, 257], F32, name="kvR")
                kvI = kvacc.tile([128, 257], F32, name="kvI")
                with tc.tile_pool(name="p1", bufs=3) as p1, \
                     tc.tile_pool(name="kfp", bufs=4) as kfp, \
                     tc.tile_pool(name="xp", bufs=3) as xp, \
                     tc.tile_pool(name="qps", bufs=1, space="PSUM") as qps, \
                     tc.tile_pool(name="kvps", bufs=1, space="PSUM") as kvps:
                    kf_hist = [None, None, None]
                    va_hist = [None, None, None, None]
                    rot_hist = [None, None, None]
                    xt_tiles = {}
                    tab_tiles = {}

                    def dma_xt(s):
                        xt = xp.tile([128, 4, 512], F8, name="xt")
                        nc.sync.dma_start(
                            xt[:], xT8[:, s * 2048:(s + 1) * 2048].rearrange("p (c t) -> p c t", c=4))
                        xt_tiles[s] = xt

                    def dma_tab(s):
                        tq = p1.tile([128, 2, 2, 512], BF16, name="tq")
                        tk = p1.tile([128, 2, 4, 256], BF16, name="tk")
                        nc.sync.dma_start(
                            tq[:, 0, :, :], qtab[:, s * 1024:(s + 1) * 1024].rearrange("p (b t) -> p b t", b=2))
                        nc.sync.dma_start(
                            tq[:, 1, :, :], qtab2[:, s * 1024:(s + 1) * 1024].rearrange("p (b t) -> p b t", b=2))
                        nc.sync.dma_start(
                            tk[:, 0, :, :], kt1[:, s * 1024:(s + 1) * 1024].rearrange("p (t d) -> p t d", t=4))
                        nc.sync.dma_start(
                            tk[:, 1, :, :], kt2[:, s * 1024:(s + 1) * 1024].rearrange("p (t d) -> p t d", t=4))
                        tab_tiles[s] = (tq, tk)

                    def emit_kv(s):
                        kf = kf_hist[s % 3]
                        va = va_hist[s % 4]
                        for t in range(4):
                            st = (s == 0 and t == 0)
                            sp = (s == nspan - 1 and t == 3)
                            nc.tensor.matmul(kvR[:], kf[:, 0, t, :], va[:, t, :],
                                             start=st, stop=sp)
                            nc.tensor.matmul(kvI[:], kf[:, 1, t, :], va[:, t, :],
                                             start=st, stop=sp)

                    # prefetch: x tiles 2 spans ahead, tables 1 span ahead
                    dma_xt(0)
                    dma_xt(1)
                    dma_tab(0)
                    for s in range(nspan):
                        if s + 2 < nspan:
                            dma_xt(s + 2)
                        if s + 1 < nspan:
                            dma_tab(s + 1)
                        xt = xt_tiles.pop(s)
                        tq, tk = tab_tiles.pop(s)

                        # q matmuls: out [128 d, 2 blk, 512 tok], DoubleRow over
                        # c-chunk pairs (bias added on the scalar eviction)
                        q_ps = qps.tile([128, 2, 512], F32, name="qp")
                        for blk in range(2):
                            for j in range(2):
                                nc.tensor.matmul(
                                    q_ps[:, blk, :],
                                    wq_t[:, 2 * j:2 * j + 2, blk * 128:(blk + 1) * 128],
                                    xt[:, 2 * j:2 * j + 2, :],
                                    start=(j == 0), stop=(j == 1), perf_mode=DR)
                        # k+v matmuls: out [128 tok, 512 (kR kI v)] per t-tile;
                        # the 4 bf16 k-bias matmuls run as one batch AFTER all
                        # DR matmuls (interleaving bf16 into the fp8 stream
                        # breaks DoubleRow packing: 216 -> 318+ ns/MM)
                        kv_ps = kvps.tile([128, 4, 512], F32, name="kvp")
                        for t in range(4):
                            for j in range(2):
                                nc.tensor.matmul(
                                    kv_ps[:, t, :],
                                    xt[:, 2 * j:2 * j + 2, t * 128:(t + 1) * 128],
                                    wkv_t[:, 2 * j:2 * j + 2, :],
                                    start=(j == 0), stop=False, perf_mode=DR)
                        for t in range(4):
                            nc.tensor.matmul(kv_ps[:, t, :], ones_t[0:1, 0:128],
                                             bk_t[:], start=False, stop=True)
                        # kv accumulation, three spans back
                        if s > 2:
                            emit_kv(s - 3)

                        # ---- evict q to SBUF (scalar, bias folded in) ----
                        q_sb = p1.tile([128, 1, 2, 512], BF16, name="qsb")
                        for blk in range(2):
                            nc.scalar.activation(q_sb[:, 0, blk, :], q_ps[:, blk, :],
                                                 AF.Identity,
                                                 bias=bq_t[:, blk:blk + 1])
                        # ---- evict k to SBUF (scalar) ----
                        k_sb = p1.tile([128, 1, 4, 256], BF16, name="ksb")
                        nc.scalar.copy(k_sb[:, 0, :, :], kv_ps[:, :, 0:256])

                        # ---- va: v * 2^-9 -> fp8 (w-scale 1/64 * v-scale 1/8)
                        va = kfp.tile([128, 4, 257], BF16, name="va")
                        if s < 4:
                            nc.vector.memset(va[:, :, 256:257], 1.0 / 128.0)
                        nc.scalar.activation(va[:, :, 0:256], kv_ps[:, :, 256:512],
                                             AF.Identity, scale=1.0 / 512.0)

                        # ---- rope products, both rotations in one op ----
                        # P[:,0] = q*[c;-s], P[:,1] = q*[s;c]; same for k
                        P = p1.tile([128, 2, 2, 512], BF16, name="P")
                        nc.vector.tensor_tensor(
                            P[:], q_sb.broadcast_to([128, 2, 2, 512]), tq[:], op=ALU.mult)
                        tkP = p1.tile([128, 2, 4, 256], BF16, name="tkP")
                        nc.vector.tensor_tensor(
                            tkP[:], k_sb.broadcast_to([128, 2, 4, 256]), tk[:], op=ALU.mult)

                        # ---- combine rope halves ----
                        # rot layout: [128, 4, 512] = qR' qI' kR' kI'
                        rot = p1.tile([128, 4, 512], BF16, name="rot")
                        nc.vector.tensor_tensor(rot[:, 0:2, :], P[:, :, 0, :],
                                                P[:, :, 1, :], op=ALU.add)
                        nc.vector.tensor_tensor(
                            rot[:, 2:4, :].rearrange("p a (t d) -> p a t d", d=128),
                            tkP[:, :, :, 0:128], tkP[:, :, :, 128:256], op=ALU.add)
                        a_t = p1.tile([128, 4, 512], BF16, name="at")
                        nc.vector.tensor_scalar(a_t[:], rot[:], 1.0, 1.0,
                                                op0=ALU.add, op1=ALU.max)
                        rot_hist[s % 3] = (rot, a_t)

                        # ---- feature map elu(x)+1 = min(exp(x), max(x+1,1)),
                        # pipelined one span back so vector never waits on exp
                        def emit_feat(sf):
                            rot_f, a_f = rot_hist[sf % 3]
                            exp_t = p1.tile([128, 4, 512], BF16, name="expt")
                            nc.scalar.activation(exp_t[:], rot_f[:], AF.Exp)
                            nc.vector.tensor_tensor(
                                q_store[:, sf, :, :],
                                exp_t[:, 0:2, :], a_f[:, 0:2, :], op=ALU.min)
                            kf = kfp.tile([128, 2, 4, 128], BF16, name="kf")
                            nc.vector.tensor_tensor(
                                kf.rearrange("p r t d -> p r (t d)"),
                                exp_t[:, 2:4, :], a_f[:, 2:4, :], op=ALU.min)
                            kf_hist[sf % 3] = kf

                        if s > 1:
                            emit_feat(s - 2)
                        va_hist[s % 4] = va
                    emit_feat(nspan - 2)
                    emit_feat(nspan - 1)
                    emit_kv(nspan - 3)
                    emit_kv(nspan - 2)
                    emit_kv(nspan - 1)

                # ---- extract block-diag kv lhsT (fp8 R/I pairs) + den lhsT ----
                nc.vector.tensor_tensor(lkv[0][:, 0, :], kvR[:, 0:128], mask0_t[:], op=ALU.mult)
                nc.vector.tensor_tensor(lkv[0][:, 1, :], kvI[:, 0:128], mask0_t[:], op=ALU.mult)
                nc.vector.tensor_tensor(lkv[1][:, 0, :], kvR[:, 128:256], mask1_t[:], op=ALU.mult)
                nc.vector.tensor_tensor(lkv[1][:, 1, :], kvI[:, 128:256], mask1_t[:], op=ALU.mult)
                for j in range(4):
                    nc.vector.tensor_tensor(
                        dm8[:, j, 0, :], kvR[:, 256:257].broadcast_to([128, 16]),
                        dmask_t[:, j, :], op=ALU.mult)
                    nc.vector.tensor_tensor(
                        dm8[:, j, 1, :], kvI[:, 256:257].broadcast_to([128, 16]),
                        dmask_t[:, j, :], op=ALU.mult)

            # ===== pass 2 =====
            with tc.tile_pool(name="p2", bufs=3) as p2, \
                 tc.tile_pool(name="zt", bufs=2) as zt, \
                 tc.tile_pool(name="dps", bufs=2, space="PSUM") as dps, \
                 tc.tile_pool(name="ops", bufs=1, space="PSUM") as ops, \
                 tc.tile_pool(name="zps", bufs=1, space="PSUM") as zps, \
                 tc.tile_pool(name="yps", bufs=2, space="PSUM") as yps:
                den_tiles = {}
                zinv_tiles = {}
                st = {}

                def stage_den(s):
                    # den for span s at partitions 4*(s%4); group of 4 spans
                    # shares one PSUM bank via span-phase mask variants
                    g, j = s // 4, s % 4
                    if j == 0:
                        den_tiles[g] = dps.tile([16, 512], F32, name="denp")
                    nc.tensor.matmul(den_tiles[g][:], dm8[:, j, 0, :],
                                     q_store[:, s, 0, :],
                                     start=(j == 0), stop=False)
                    nc.tensor.matmul(den_tiles[g][:], dm8[:, j, 1, :],
                                     q_store[:, s, 1, :],
                                     start=False, stop=(j == 3))

                def stage_zinv(g):
                    # 1/z = exp(-ln(z)) for 4 spans at once
                    den_ps = den_tiles.pop(g)
                    lnz = p2.tile([16, 512], F32, name="lnz")
                    nc.scalar.activation(lnz[:], den_ps[:], AF.Ln)
                    zi = zt.tile([16, 512], BF16, name="zinv")
                    nc.scalar.activation(zi[:], lnz[:], AF.Exp, scale=-1.0)
                    zinv_tiles[g] = zi

                def stage_a(s):
                    g, j = s // 4, s % 4
                    out_ps = ops.tile([128, 2, 512], F32, name="outp")
                    for i in range(2):
                        nc.tensor.matmul(out_ps[:, i, :], lkv[i][:, 0, :],
                                         q_store[:, s, 0, :],
                                         start=True, stop=False)
                        nc.tensor.matmul(out_ps[:, i, :], lkv[i][:, 1, :],
                                         q_store[:, s, 1, :],
                                         start=False, stop=True)
                    zb_ps = zps.tile([128, 2, 512], F32, name="zbp")
                    zi = zinv_tiles[g]
                    for i in range(2):
                        nc.tensor.matmul(zb_ps[:, i, :], sel_t[:, j, i, :],
                                         zi[:], start=True, stop=True)
                    zb_sb = p2.tile([128, 2, 512], BF16, name="zbs")
                    nc.scalar.copy(zb_sb[:], zb_ps[:])
                    outT = p2.tile([128, 2, 512], BF16, name="outT")
                    nc.vector.tensor_tensor(outT[:], out_ps[:], zb_sb[:], op=ALU.mult)
                    st[s] = outT

                def stage_c(s):
                    outT = st.pop(s)
                    for t in range(4):
                        y_ps = yps.tile([128, 512], F32, name="yp")
                        for i in range(2):
                            nc.tensor.matmul(y_ps[:],
                                             outT[:, i, t * 128:(t + 1) * 128],
                                             wp_t[:, i, :], start=(i == 0),
                                             stop=(i == 1))
                        y_sb = p2.tile([128, 512], BF16, name="ysb")
                        if t % 2 == 0:
                            nc.scalar.copy(y_sb[:], y_ps[:])
                        else:
                            nc.vector.tensor_copy(y_sb[:], y_ps[:])
                        nc.sync.dma_start(
                            y[s * 512 + t * 128:s * 512 + (t + 1) * 128, :],
                            y_sb[:])

                for s in range(8):
                    stage_den(s)
                stage_zinv(0)
                for s in range(nspan):
                    if s >= 2:
                        stage_c(s - 2)
                    if s + 8 < nspan:
                        stage_den(s + 8)
                    if s % 4 == 0 and s + 4 < nspan:
                        stage_zinv(s // 4 + 1)
                    stage_a(s)
                stage_c(nspan - 2)
                stage_c(nspan - 1)

    return nc


_NC_CACHE = {}


def _get_nc(n_tok):
    if n_tok not in _NC_CACHE:
        _NC_CACHE[n_tok] = _build_nc(n_tok)
    return _NC_CACHE[n_tok]


def _bf(a):
    return np.ascontiguousarray(np.asarray(a, dtype=np.float32)).astype(ml_dtypes.bfloat16)


def _f8(a, scale=1.0):
    return np.ascontiguousarray(
        np.asarray(a, dtype=np.float32) * scale).astype(ml_dtypes.float8_e4m3)


_TABLES_CACHE = {}


def _tables(nspan):
    """Per-span rope tables (x1/64 to undo the x64 fp8 weight scale).

    qtab [128 d, nspan, 2, 512]: [cos ; -sin] in qT layout.
    ktc/kts [128 tok, nspan, 4 t, 128 d]: cos/sin in token layout.
    """
    if nspan in _TABLES_CACHE:
        return _TABLES_CACHE[nspan]
    j = np.arange(16)
    freqs = (1.0 / (THETA ** (4.0 * j / HD))).astype(np.float64)
    fcol = np.tile(freqs, 4)                      # [64] head-major 4h x 16f
    tx = np.arange(128).astype(np.float64)
    angx = np.outer(fcol, tx)                     # [64 f, 128 tx]
    rows = np.arange(nspan * 4).astype(np.float64)
    angy = np.outer(fcol, rows)                   # [64 f, nspan*4]

    # q tables [128, nspan, 4, 128]
    qc = np.empty((128, nspan, 4, 128), np.float32)
    qs_ = np.empty((128, nspan, 4, 128), np.float32)
    qc[0:64] = np.cos(angx)[:, None, None, :]
    qs_[0:64] = np.sin(angx)[:, None, None, :]
    qc[64:128] = np.cos(angy).reshape(64, nspan, 4, 1)
    qs_[64:128] = np.sin(angy).reshape(64, nspan, 4, 1)
    qcf = qc.reshape(128, nspan, 1, 512)
    qsf = qs_.reshape(128, nspan, 1, 512)
    qtab = np.concatenate([qcf, -qsf], axis=2).reshape(128, nspan * 1024) / 64.0
    qtab2 = np.concatenate([qsf, qcf], axis=2).reshape(128, nspan * 1024) / 64.0

    # k tables [128 tok(part), nspan, 4 t, 128 d], paired [c|-s] and [s|c]
    kc = np.empty((128, nspan, 4, 128), np.float32)
    ks = np.empty((128, nspan, 4, 128), np.float32)
    kc[:, :, :, 0:64] = np.cos(angx).T[:, None, None, :]
    ks[:, :, :, 0:64] = np.sin(angx).T[:, None, None, :]
    kc[:, :, :, 64:128] = np.cos(angy).T.reshape(1, nspan, 4, 64)
    ks[:, :, :, 64:128] = np.sin(angy).T.reshape(1, nspan, 4, 64)
    kt1 = np.concatenate([kc, -ks], axis=3).reshape(128, nspan * 1024) / 64.0
    kt2 = np.concatenate([ks, kc], axis=3).reshape(128, nspan * 1024) / 64.0
    out = (_bf(qtab), _bf(qtab2), _bf(kt1), _bf(kt2))
    _TABLES_CACHE[nspan] = out
    return out


def kernel(x, w_qkv, b_qkv, w_proj, b_proj, height, width):
    x = np.asarray(x); w_qkv = np.asarray(w_qkv); b_qkv = np.asarray(b_qkv)
    w_proj = np.asarray(w_proj); b_proj = np.asarray(b_proj)
    b, n, c = x.shape
    nc = _get_nc(n)
    qtab, qtab2, kt1, kt2 = _tables(n // 512)

    # span-phase variants: variant j reads zinv rows 4j..4j+3 / writes den
    # columns 4j..4j+3 (4 spans share one den PSUM bank / zinv tile)
    sel = np.zeros((4, 16, 2, 128), np.float32)
    for j in range(4):
        for i in range(2):
            for hf in range(2):
                sel[j, 4 * j + 2 * i + hf, i, 64 * hf:64 * hf + 64] = 1.0
    # head of partition p in the d layout: (p % 64) // 16
    hop = (np.arange(128) % 64) // 16
    mask0 = (hop[:, None] == (np.arange(128) // 64)[None, :]).astype(np.float32)
    mask1 = (hop[:, None] == (2 + np.arange(128) // 64)[None, :]).astype(np.float32)
    dmask = np.zeros((4, 128, 16), np.float32)
    for j in range(4):
        for h in range(4):
            dmask[j, :, 4 * j + h] = (hop == h)

    in_maps = []
    for core in range(8):
        bi, hg = core // 2, core % 2
        heads = [hg * NH + j for j in range(NH)]
        q0 = [h * HD + 2 * j for h in heads for j in range(16)] + \
             [h * HD + 32 + 2 * j for h in heads for j in range(16)]
        q1 = [cc + 1 for cc in q0]
        kR = [512 + cc for cc in q0]
        kI = [512 + cc for cc in q1]
        vc = [1024 + h * HD + e for h in heads for e in range(HD)]
        wq_cols = q0 + q1
        wkv_cols = kR + kI + vc
        bk = np.concatenate([b_qkv[kR + kI] * 64.0, np.zeros(256, np.float32)])
        in_maps.append({
            "xT8": _f8(x[bi].T.reshape(4, 128, n // 512, 512).transpose(1, 2, 0, 3).reshape(128, -1)),
            "w_q": _f8(w_qkv[:, wq_cols], 64.0).reshape(4, 128, 256),
            "w_kv": _f8(w_qkv[:, wkv_cols], 64.0).reshape(4, 128, 512),
            "bq": np.stack([b_qkv[q0], b_qkv[q1]], axis=1).astype(np.float32) * 64.0,
            "bkrow": _bf(bk)[None, :],
            "wp": _bf(np.stack([w_proj[hg * 256:hg * 256 + 128, :],
                                w_proj[hg * 256 + 128:hg * 256 + 256, :]])),
            "qtab": qtab, "qtab2": qtab2, "kt1": kt1, "kt2": kt2,
            "sel": _bf(sel),
            "mask0": _bf(mask0), "mask1": _bf(mask1), "dmask": _bf(dmask),
        })
    res = run_bass_kernel_spmd(nc, in_maps, list(range(8)), trace=False)
    bias_eff = (b_proj.astype(np.float32)
                + b_qkv[1024:].astype(np.float32) @ w_proj.astype(np.float32))
    out = np.empty((b, n, c), np.float32)
    for bi in range(b):
        out[bi] = ((res.results[2 * bi]["y"].astype(np.float32)
                    + res.results[2 * bi + 1]["y"].astype(np.float32)) * (2.0 ** -4)
                   + bias_eff[None, :])
    return out


# revision 4
# speedup vs baseline: 1.0457x; 1.0271x over previous
import sys

sys.path.insert(0, "/opt/trn_rl_repo")
import numpy as np
import ml_dtypes
import concourse.bass as bass
import concourse.mybir as mybir
import concourse.tile as tile
from concourse.bass_utils import run_bass_kernel_spmd

F32 = mybir.dt.float32
BF16 = mybir.dt.bfloat16
F8 = mybir.dt.float8e4
AF = mybir.ActivationFunctionType
ALU = mybir.AluOpType
DR = mybir.MatmulPerfMode.DoubleRow

C = 512
NH = 4          # heads per core (8 global, split in 2 groups of 4)
HD = 64
THETA = 10.0

import json as _json
import concourse.bass2jax as _b2j
import concourse.bass_utils as _bu

_ORIG_COMPILE = _bu.compile_bir_kernel


def _patched_compile_bir_kernel(bir_json, tmpdir, neff_name="file.neff"):
    """Walrus rejects instructions whose sync waits+updates exceed 2.
    Rewrite the BIR: move excess waits onto inserted same-engine Drains."""
    d = _json.loads(bir_json)
    for fn in d.get("functions", []):
        for b in fn.get("blocks", []):
            out = []
            for i in b.get("instructions", []):
                si = i.get("sync_info")
                if si:
                    ow = si.get("on_wait") or []
                    ou = si.get("on_update") or []
                    cap = 1 if i.get("opcode") in ("Drain", "Ldweights") else 2
                    budget = cap - len(ou)
                    if len(ow) > budget:
                        keep = ow[-budget:] if budget > 0 else []
                        extra = ow[:-budget] if budget > 0 else ow
                        for ci, w in enumerate(extra):
                            out.append({
                                "debug": i.get("debug", 0),
                                "engine": i["engine"],
                                "ins": [], "outs": [],
                                "name": f"{i['name']}sw{ci}",
                                "opcode": "Drain",
                                "sync_info": {"on_update": [],
                                              "on_wait": [w]},
                            })
                        si["on_wait"] = keep
                out.append(i)
            b["instructions"] = out
    return _ORIG_COMPILE(_json.dumps(d).encode(), tmpdir, neff_name=neff_name)


_bu.compile_bir_kernel = _patched_compile_bir_kernel
_b2j.compile_bir_kernel = _patched_compile_bir_kernel


def _build_nc(n_tok):
    nspan = n_tok // 512
    nc = bass.Bass()
    xT8 = nc.declare_dram_parameter("xT8", [128, nspan * 2048], F8, isOutput=False)
    w_q = nc.declare_dram_parameter("w_q", [4, 128, 256], F8, isOutput=False)
    w_kv = nc.declare_dram_parameter("w_kv", [4, 128, 512], F8, isOutput=False)
    bq = nc.declare_dram_parameter("bq", [128, 2], F32, isOutput=False)
    bkrow = nc.declare_dram_parameter("bkrow", [1, 512], BF16, isOutput=False)
    wp = nc.declare_dram_parameter("wp", [2, 128, 512], BF16, isOutput=False)
    # per-span rope tables: q [c;-s] and [s;c]; k paired [c|-s] and [s|c]
    # (all pre-scaled 1/64 to undo the x64 fp8 weight scale)
    qtab = nc.declare_dram_parameter("qtab", [128, nspan * 1024], BF16, isOutput=False)
    qtab2 = nc.declare_dram_parameter("qtab2", [128, nspan * 1024], BF16, isOutput=False)
    kt1 = nc.declare_dram_parameter("kt1", [128, nspan * 1024], BF16, isOutput=False)
    kt2 = nc.declare_dram_parameter("kt2", [128, nspan * 1024], BF16, isOutput=False)
    sel = nc.declare_dram_parameter("sel", [4, 16, 2, 128], BF16, isOutput=False)
    mask0 = nc.declare_dram_parameter("mask0", [128, 128], BF16, isOutput=False)
    mask1 = nc.declare_dram_parameter("mask1", [128, 128], BF16, isOutput=False)
    dmask = nc.declare_dram_parameter("dmask", [4, 128, 16], BF16, isOutput=False)
    y = nc.declare_dram_parameter("y", [n_tok, 512], BF16, isOutput=True)

    with nc.allow_low_precision(reason="fp8 pipeline by design"), tile.TileContext(nc) as tc:
        with tc.tile_pool(name="wpool", bufs=1) as wpool, \
             tc.tile_pool(name="store", bufs=1) as store:
            # ---- persistent tiles ----
            wq_t = wpool.tile([128, 4, 256], F8, name="wq")
            wkv_t = wpool.tile([128, 4, 512], F8, name="wkv")
            bq_t = wpool.tile([128, 2], F32, name="bq")
            bk_t = wpool.tile([1, 512], BF16, name="bk")
            wp_t = wpool.tile([128, 2, 512], BF16, name="wp")
            sel_t = wpool.tile([16, 4, 2, 128], BF16, name="sel")
            mask0_t = wpool.tile([128, 128], BF16, name="mask0")
            mask1_t = wpool.tile([128, 128], BF16, name="mask1")
            dmask_t = wpool.tile([128, 4, 16], BF16, name="dmask")
            ones_t = wpool.tile([1, 512], BF16, name="ones")

            nc.sync.dma_start(wq_t[:], w_q.rearrange("c p d -> p c d"))
            nc.sync.dma_start(wkv_t[:], w_kv.rearrange("c p d -> p c d"))
            nc.sync.dma_start(bq_t[:], bq[:])
            nc.sync.dma_start(bk_t[:], bkrow[:])
            nc.vector.memset(ones_t[:], 1.0)

            def load_pass2_consts():
                # deferred off the startup DMA burst; only needed after pass 1
                nc.sync.dma_start(wp_t[:], wp.rearrange("e p c -> p e c"))
                nc.sync.dma_start(sel_t[:], sel.rearrange("j p i d -> p j i d"))
                nc.sync.dma_start(mask0_t[:], mask0[:])
                nc.sync.dma_start(mask1_t[:], mask1[:])
                nc.sync.dma_start(dmask_t[:], dmask.rearrange("j p c -> p j c"))

            # q features for the whole sequence: [128 d, 2 (R/I), n_tok] fp8
            # span-major so each span's [128, 2, 512] slice is a dense window
            # (a 2*n_tok-strided middle dim defeats DoubleRow rhs packing)
            q_store = store.tile([128, n_tok // 512, 2, 512], BF16, name="qs")

            # kv lhsT (fp8, DoubleRow R/I pairs) + den lhsT (4 span-phase
            # variants: variant j writes only den columns 4j..4j+3)
            lkv = [wpool.tile([128, 2, 128], BF16, name=f"lkv{i}") for i in range(2)]
            dm8 = wpool.tile([128, 4, 2, 16], BF16, name="dm8")

            # ================ pass 1 ================
            with tc.tile_pool(name="kvacc", bufs=1, space="PSUM") as kvacc:
                kvR = kvacc.tile([128, 257], F32, name="kvR")
                kvI = kvacc.tile([128, 257], F32, name="kvI")
                with tc.tile_pool(name="p1", bufs=3) as p1, \
                     tc.tile_pool(name="kfp", bufs=4) as kfp, \
                     tc.tile_pool(name="xp", bufs=3) as xp, \
                     tc.tile_pool(name="qps", bufs=1, space="PSUM") as qps, \
                     tc.tile_pool(name="kvps", bufs=1, space="PSUM") as kvps:
                    kf_hist = [None, None, None]
                    va_hist = [None, None, None, None]
                    rot_hist = [None, None, None]
                    xt_tiles = {}
                    tab_tiles = {}

                    def dma_xt(s):
                        xt = xp.tile([128, 4, 512], F8, name="xt")
                        nc.sync.dma_start(
                            xt[:], xT8[:, s * 2048:(s + 1) * 2048].rearrange("p (c t) -> p c t", c=4))
                        xt_tiles[s] = xt

                    def dma_tab(s):
                        tq = p1.tile([128, 2, 2, 512], BF16, name="tq")
                        tk = p1.tile([128, 2, 4, 256], BF16, name="tk")
                        nc.sync.dma_start(
                            tq[:, 0, :, :], qtab[:, s * 1024:(s + 1) * 1024].rearrange("p (b t) -> p b t", b=2))
                        nc.sync.dma_start(
                            tq[:, 1, :, :], qtab2[:, s * 1024:(s + 1) * 1024].rearrange("p (b t) -> p b t", b=2))
                        nc.sync.dma_start(
                            tk[:, 0, :, :], kt1[:, s * 1024:(s + 1) * 1024].rearrange("p (t d) -> p t d", t=4))
                        nc.sync.dma_start(
                            tk[:, 1, :, :], kt2[:, s * 1024:(s + 1) * 1024].rearrange("p (t d) -> p t d", t=4))
                        tab_tiles[s] = (tq, tk)

                    def emit_kv(s):
                        kf = kf_hist[s % 3]
                        va = va_hist[s % 4]
                        for t in range(4):
                            st = (s == 0 and t == 0)
                            sp = (s == nspan - 1 and t == 3)
                            nc.tensor.matmul(kvR[:], kf[:, 0, t, :], va[:, t, :],
                                             start=st, stop=sp)
                            nc.tensor.matmul(kvI[:], kf[:, 1, t, :], va[:, t, :],
                                             start=st, stop=sp)

                    # prefetch: x tiles 2 spans ahead, tables 1 span ahead
                    dma_xt(0)
                    dma_xt(1)
                    dma_tab(0)
                    for s in range(nspan):
                        if s + 2 < nspan:
                            dma_xt(s + 2)
                        if s + 1 < nspan:
                            dma_tab(s + 1)
                        xt = xt_tiles.pop(s)
                        tq, tk = tab_tiles.pop(s)

                        # q matmuls: out [128 d, 2 blk, 512 tok], DoubleRow over
                        # c-chunk pairs (bias added on the scalar eviction)
                        q_ps = qps.tile([128, 2, 512], F32, name="qp")
                        for blk in range(2):
                            for j in range(2):
                                nc.tensor.matmul(
                                    q_ps[:, blk, :],
                                    wq_t[:, 2 * j:2 * j + 2, blk * 128:(blk + 1) * 128],
                                    xt[:, 2 * j:2 * j + 2, :],
                                    start=(j == 0), stop=(j == 1), perf_mode=DR)
                        # k+v matmuls: out [128 tok, 512 (kR kI v)] per t-tile;
                        # the 4 bf16 k-bias matmuls run as one batch AFTER all
                        # DR matmuls (interleaving bf16 into the fp8 stream
                        # breaks DoubleRow packing: 216 -> 318+ ns/MM)
                        kv_ps = kvps.tile([128, 4, 512], F32, name="kvp")
                        for t in range(4):
                            for j in range(2):
                                nc.tensor.matmul(
                                    kv_ps[:, t, :],
                                    xt[:, 2 * j:2 * j + 2, t * 128:(t + 1) * 128],
                                    wkv_t[:, 2 * j:2 * j + 2, :],
                                    start=(j == 0), stop=False, perf_mode=DR)
                        for t in range(4):
                            nc.tensor.matmul(kv_ps[:, t, :], ones_t[0:1, 0:128],
                                             bk_t[:], start=False, stop=True)
                        if s == 2:
                            load_pass2_consts()
                        # kv accumulation, three spans back
                        if s > 2:
                            emit_kv(s - 3)

                        # ---- evict q to SBUF (scalar, bias folded in) ----
                        q_sb = p1.tile([128, 1, 2, 512], BF16, name="qsb")
                        for blk in range(2):
                            nc.scalar.activation(q_sb[:, 0, blk, :], q_ps[:, blk, :],
                                                 AF.Identity,
                                                 bias=bq_t[:, blk:blk + 1])
                        # ---- evict k to SBUF (scalar) ----
                        k_sb = p1.tile([128, 1, 4, 256], BF16, name="ksb")
                        nc.scalar.copy(k_sb[:, 0, :, :], kv_ps[:, :, 0:256])

                        # ---- va: v * 2^-9 -> fp8 (w-scale 1/64 * v-scale 1/8)
                        va = kfp.tile([128, 4, 257], BF16, name="va")
                        if s < 4:
                            nc.vector.memset(va[:, :, 256:257], 1.0 / 128.0)
                        nc.scalar.activation(va[:, :, 0:256], kv_ps[:, :, 256:512],
                                             AF.Identity, scale=1.0 / 512.0)

                        # ---- rope products, both rotations in one op ----
                        # P[:,0] = q*[c;-s], P[:,1] = q*[s;c]; same for k
                        P = p1.tile([128, 2, 2, 512], BF16, name="P")
                        nc.vector.tensor_tensor(
                            P[:], q_sb.broadcast_to([128, 2, 2, 512]), tq[:], op=ALU.mult)
                        tkP = p1.tile([128, 2, 4, 256], BF16, name="tkP")
                        nc.vector.tensor_tensor(
                            tkP[:], k_sb.broadcast_to([128, 2, 4, 256]), tk[:], op=ALU.mult)

                        # ---- combine rope halves ----
                        # rot layout: [128, 4, 512] = qR' qI' kR' kI'
                        rot = p1.tile([128, 4, 512], BF16, name="rot")
                        nc.vector.tensor_tensor(rot[:, 0:2, :], P[:, :, 0, :],
                                                P[:, :, 1, :], op=ALU.add)
                        nc.vector.tensor_tensor(
                            rot[:, 2:4, :].rearrange("p a (t d) -> p a t d", d=128),
                            tkP[:, :, :, 0:128], tkP[:, :, :, 128:256], op=ALU.add)
                        a_t = p1.tile([128, 4, 512], BF16, name="at")
                        nc.vector.tensor_scalar(a_t[:], rot[:], 1.0, 1.0,
                                                op0=ALU.add, op1=ALU.max)
                        rot_hist[s % 3] = (rot, a_t)

                        # ---- feature map elu(x)+1 = min(exp(x), max(x+1,1)),
                        # pipelined one span back so vector never waits on exp
                        def emit_feat(sf):
                            rot_f, a_f = rot_hist[sf % 3]
                            exp_t = p1.tile([128, 4, 512], BF16, name="expt")
                            nc.scalar.activation(exp_t[:], rot_f[:], AF.Exp)
                            nc.vector.tensor_tensor(
                                q_store[:, sf, :, :],
                                exp_t[:, 0:2, :], a_f[:, 0:2, :], op=ALU.min)
                            kf = kfp.tile([128, 2, 4, 128], BF16, name="kf")
                            nc.vector.tensor_tensor(
                                kf.rearrange("p r t d -> p r (t d)"),
                                exp_t[:, 2:4, :], a_f[:, 2:4, :], op=ALU.min)
                            kf_hist[sf % 3] = kf

                        if s > 1:
                            emit_feat(s - 2)
                        va_hist[s % 4] = va
                    emit_feat(nspan - 2)
                    emit_feat(nspan - 1)
                    emit_kv(nspan - 3)
                    emit_kv(nspan - 2)
                    emit_kv(nspan - 1)

                # ---- extract block-diag kv lhsT (fp8 R/I pairs) + den lhsT ----
                nc.vector.tensor_tensor(lkv[0][:, 0, :], kvR[:, 0:128], mask0_t[:], op=ALU.mult)
                nc.vector.tensor_tensor(lkv[0][:, 1, :], kvI[:, 0:128], mask0_t[:], op=ALU.mult)
                nc.vector.tensor_tensor(lkv[1][:, 0, :], kvR[:, 128:256], mask1_t[:], op=ALU.mult)
                nc.vector.tensor_tensor(lkv[1][:, 1, :], kvI[:, 128:256], mask1_t[:], op=ALU.mult)
                for j in range(4):
                    nc.vector.tensor_tensor(
                        dm8[:, j, 0, :], kvR[:, 256:257].broadcast_to([128, 16]),
                        dmask_t[:, j, :], op=ALU.mult)
                    nc.vector.tensor_tensor(
                        dm8[:, j, 1, :], kvI[:, 256:257].broadcast_to([128, 16]),
                        dmask_t[:, j, :], op=ALU.mult)

            # ===== pass 2 =====
            with tc.tile_pool(name="p2", bufs=3) as p2, \
                 tc.tile_pool(name="zt", bufs=2) as zt, \
                 tc.tile_pool(name="dps", bufs=2, space="PSUM") as dps, \
                 tc.tile_pool(name="ops", bufs=1, space="PSUM") as ops, \
                 tc.tile_pool(name="zps", bufs=1, space="PSUM") as zps, \
                 tc.tile_pool(name="yps", bufs=2, space="PSUM") as yps:
                den_tiles = {}
                zinv_tiles = {}
                st = {}

                def stage_den(s):
                    # den for span s at partitions 4*(s%4); group of 4 spans
                    # shares one PSUM bank via span-phase mask variants
                    g, j = s // 4, s % 4
                    if j == 0:
                        den_tiles[g] = dps.tile([16, 512], F32, name="denp")
                    nc.tensor.matmul(den_tiles[g][:], dm8[:, j, 0, :],
                                     q_store[:, s, 0, :],
                                     start=(j == 0), stop=False)
                    nc.tensor.matmul(den_tiles[g][:], dm8[:, j, 1, :],
                                     q_store[:, s, 1, :],
                                     start=False, stop=(j == 3))

                def stage_zinv(g):
                    # 1/z = exp(-ln(z)) for 4 spans at once
                    den_ps = den_tiles.pop(g)
                    lnz = p2.tile([16, 512], F32, name="lnz")
                    nc.scalar.activation(lnz[:], den_ps[:], AF.Ln)
                    zi = zt.tile([16, 512], BF16, name="zinv")
                    nc.scalar.activation(zi[:], lnz[:], AF.Exp, scale=-1.0)
                    zinv_tiles[g] = zi

                def stage_a(s):
                    g, j = s // 4, s % 4
                    zb_ps = zps.tile([128, 2, 512], F32, name="zbp")
                    zi = zinv_tiles[g]
                    for i in range(2):
                        nc.tensor.matmul(zb_ps[:, i, :], sel_t[:, j, i, :],
                                         zi[:], start=True, stop=True)
                    zb_sb = p2.tile([128, 2, 512], BF16, name="zbs")
                    nc.scalar.copy(zb_sb[:], zb_ps[:])
                    out_ps = ops.tile([128, 2, 512], F32, name="outp")
                    for i in range(2):
                        nc.tensor.matmul(out_ps[:, i, :], lkv[i][:, 0, :],
                                         q_store[:, s, 0, :],
                                         start=True, stop=False)
                        nc.tensor.matmul(out_ps[:, i, :], lkv[i][:, 1, :],
                                         q_store[:, s, 1, :],
                                         start=False, stop=True)
                    outT = p2.tile([128, 2, 512], BF16, name="outT")
                    nc.vector.tensor_tensor(outT[:], out_ps[:], zb_sb[:], op=ALU.mult)
                    st[s] = outT

                def stage_c(s):
                    outT = st.pop(s)
                    for t in range(4):
                        y_ps = yps.tile([128, 512], F32, name="yp")
                        for i in range(2):
                            nc.tensor.matmul(y_ps[:],
                                             outT[:, i, t * 128:(t + 1) * 128],
                                             wp_t[:, i, :], start=(i == 0),
                                             stop=(i == 1))
                        y_sb = p2.tile([128, 512], BF16, name="ysb")
                        if t % 2 == 0:
                            nc.scalar.copy(y_sb[:], y_ps[:])
                        else:
                            nc.vector.tensor_copy(y_sb[:], y_ps[:])
                        nc.sync.dma_start(
                            y[s * 512 + t * 128:s * 512 + (t + 1) * 128, :],
                            y_sb[:])

                for s in range(8):
                    stage_den(s)
                stage_zinv(0)
                for s in range(nspan):
                    if s >= 2:
                        stage_c(s - 2)
                    if s + 8 < nspan:
                        stage_den(s + 8)
                    if s % 4 == 0 and s + 4 < nspan:
                        stage_zinv(s // 4 + 1)
                    stage_a(s)
                stage_c(nspan - 2)
                stage_c(nspan - 1)

    return nc


_NC_CACHE = {}


def _get_nc(n_tok):
    if n_tok not in _NC_CACHE:
        _NC_CACHE[n_tok] = _build_nc(n_tok)
    return _NC_CACHE[n_tok]


def _bf(a):
    return np.ascontiguousarray(np.asarray(a, dtype=np.float32)).astype(ml_dtypes.bfloat16)


def _f8(a, scale=1.0):
    return np.ascontiguousarray(
        np.asarray(a, dtype=np.float32) * scale).astype(ml_dtypes.float8_e4m3)


_TABLES_CACHE = {}


def _tables(nspan):
    """Per-span rope tables (x1/64 to undo the x64 fp8 weight scale).

    qtab [128 d, nspan, 2, 512]: [cos ; -sin] in qT layout.
    ktc/kts [128 tok, nspan, 4 t, 128 d]: cos/sin in token layout.
    """
    if nspan in _TABLES_CACHE:
        return _TABLES_CACHE[nspan]
    j = np.arange(16)
    freqs = (1.0 / (THETA ** (4.0 * j / HD))).astype(np.float64)
    fcol = np.tile(freqs, 4)                      # [64] head-major 4h x 16f
    tx = np.arange(128).astype(np.float64)
    angx = np.outer(fcol, tx)                     # [64 f, 128 tx]
    rows = np.arange(nspan * 4).astype(np.float64)
    angy = np.outer(fcol, rows)                   # [64 f, nspan*4]

    # q tables [128, nspan, 4, 128]
    qc = np.empty((128, nspan, 4, 128), np.float32)
    qs_ = np.empty((128, nspan, 4, 128), np.float32)
    qc[0:64] = np.cos(angx)[:, None, None, :]
    qs_[0:64] = np.sin(angx)[:, None, None, :]
    qc[64:128] = np.cos(angy).reshape(64, nspan, 4, 1)
    qs_[64:128] = np.sin(angy).reshape(64, nspan, 4, 1)
    qcf = qc.reshape(128, nspan, 1, 512)
    qsf = qs_.reshape(128, nspan, 1, 512)
    qtab = np.concatenate([qcf, -qsf], axis=2).reshape(128, nspan * 1024) / 64.0
    qtab2 = np.concatenate([qsf, qcf], axis=2).reshape(128, nspan * 1024) / 64.0

    # k tables [128 tok(part), nspan, 4 t, 128 d], paired [c|-s] and [s|c]
    kc = np.empty((128, nspan, 4, 128), np.float32)
    ks = np.empty((128, nspan, 4, 128), np.float32)
    kc[:, :, :, 0:64] = np.cos(angx).T[:, None, None, :]
    ks[:, :, :, 0:64] = np.sin(angx).T[:, None, None, :]
    kc[:, :, :, 64:128] = np.cos(angy).T.reshape(1, nspan, 4, 64)
    ks[:, :, :, 64:128] = np.sin(angy).T.reshape(1, nspan, 4, 64)
    kt1 = np.concatenate([kc, -ks], axis=3).reshape(128, nspan * 1024) / 64.0
    kt2 = np.concatenate([ks, kc], axis=3).reshape(128, nspan * 1024) / 64.0
    out = (_bf(qtab), _bf(qtab2), _bf(kt1), _bf(kt2))
    _TABLES_CACHE[nspan] = out
    return out


def kernel(x, w_qkv, b_qkv, w_proj, b_proj, height, width):
    x = np.asarray(x); w_qkv = np.asarray(w_qkv); b_qkv = np.asarray(b_qkv)
    w_proj = np.asarray(w_proj); b_proj = np.asarray(b_proj)
    b, n, c = x.shape
    nc = _get_nc(n)
    qtab, qtab2, kt1, kt2 = _tables(n // 512)

    # span-phase variants: variant j reads zinv rows 4j..4j+3 / writes den
    # columns 4j..4j+3 (4 spans share one den PSUM bank / zinv tile)
    sel = np.zeros((4, 16, 2, 128), np.float32)
    for j in range(4):
        for i in range(2):
            for hf in range(2):
                sel[j, 4 * j + 2 * i + hf, i, 64 * hf:64 * hf + 64] = 1.0
    # head of partition p in the d layout: (p % 64) // 16
    hop = (np.arange(128) % 64) // 16
    mask0 = (hop[:, None] == (np.arange(128) // 64)[None, :]).astype(np.float32)
    mask1 = (hop[:, None] == (2 + np.arange(128) // 64)[None, :]).astype(np.float32)
    dmask = np.zeros((4, 128, 16), np.float32)
    for j in range(4):
        for h in range(4):
            dmask[j, :, 4 * j + h] = (hop == h)

    in_maps = []
    for core in range(8):
        bi, hg = core // 2, core % 2
        heads = [hg * NH + j for j in range(NH)]
        q0 = [h * HD + 2 * j for h in heads for j in range(16)] + \
             [h * HD + 32 + 2 * j for h in heads for j in range(16)]
        q1 = [cc + 1 for cc in q0]
        kR = [512 + cc for cc in q0]
        kI = [512 + cc for cc in q1]
        vc = [1024 + h * HD + e for h in heads for e in range(HD)]
        wq_cols = q0 + q1
        wkv_cols = kR + kI + vc
        bk = np.concatenate([b_qkv[kR + kI] * 64.0, np.zeros(256, np.float32)])
        in_maps.append({
            "xT8": _f8(x[bi].T.reshape(4, 128, n // 512, 512).transpose(1, 2, 0, 3).reshape(128, -1)),
            "w_q": _f8(w_qkv[:, wq_cols], 64.0).reshape(4, 128, 256),
            "w_kv": _f8(w_qkv[:, wkv_cols], 64.0).reshape(4, 128, 512),
            "bq": np.stack([b_qkv[q0], b_qkv[q1]], axis=1).astype(np.float32) * 64.0,
            "bkrow": _bf(bk)[None, :],
            "wp": _bf(np.stack([w_proj[hg * 256:hg * 256 + 128, :],
                                w_proj[hg * 256 + 128:hg * 256 + 256, :]])),
            "qtab": qtab, "qtab2": qtab2, "kt1": kt1, "kt2": kt2,
            "sel": _bf(sel),
            "mask0": _bf(mask0), "mask1": _bf(mask1), "dmask": _bf(dmask),
        })
    res = run_bass_kernel_spmd(nc, in_maps, list(range(8)), trace=False)
    bias_eff = (b_proj.astype(np.float32)
                + b_qkv[1024:].astype(np.float32) @ w_proj.astype(np.float32))
    out = np.empty((b, n, c), np.float32)
    for bi in range(b):
        out[bi] = ((res.results[2 * bi]["y"].astype(np.float32)
                    + res.results[2 * bi + 1]["y"].astype(np.float32)) * (2.0 ** -4)
                   + bias_eff[None, :])
    return out
